# revision 14
# baseline (speedup 1.0000x reference)
"""Trainium2 Bass kernel for ConvNet forward (conv7x7s3 -> sq -> fc -> sq -> fc).

Strategy: pure data parallel over 8 NeuronCores (2048 samples each).
The conv is lowered to a block-sparse dense matrix applied via fp16 matmuls
with batch as the moving free dim (N=512). Host pre-transposes x to
feature-major fp16 layout (halves HBM traffic vs fp32; the fp32 PSUM
accumulate keeps the error ~1e-3, well under the 2e-2 gate). Used features
are packed region-major (rows 0-8 | 9-12 | 13-17 | 18-21 | 22-30 of the 31
used rows) so each of the 3 conv output groups reads a contiguous chunk
range: 30 matmul pushes/tile instead of 37 for channel-major packing.
fc1 weights are permuted to match the conv-output row grouping; the
[10, 2048] per-core output is transposed back to [B, 10] on the host.
"""

import numpy as np

for _p in ("/opt/trn_rl_repo", "/root/.axon_site/_ro/trn_rl_repo"):
    try:
        import concourse  # noqa: F401
        break
    except ImportError:
        import sys
        if _p not in sys.path:
            sys.path.insert(0, _p)

# network constants
KERNEL, STRIDE = 7, 3
C_IN, C_OUT = 3, 4
HIDDEN, OUTPUT = 64, 10
H_OUT = 9                      # (32-7)//3 + 1
B_TOT, N_CORES = 16384, 8
B_CORE = B_TOT // N_CORES      # 2048
N_TILE = 512                   # batch tile (matmul moving free dim)
T_TILES = B_CORE // N_TILE     # 4
M_GROUP = 3 * C_OUT * H_OUT    # 108 outputs per i-group (3 rows x 4 ch x 9 cols)

# Only 31x31 of each 32x32 input image is read by the conv (stride 3, k=7).
# i-group g in {0,1,2} covers output rows i in {3g,3g+1,3g+2} and needs input
# rows r in [9g, 9g+12]. Pack the 3*31*31 = 2883 used features region-major —
# rows [0,9) g0-only | [9,13) shared g0/g1 | [13,18) g1-only | [18,22) shared
# g1/g2 | [22,31) g2-only — so each group's features are one contiguous span
# and its chunk list is a contiguous range (30 total pushes, near the 23-chunk
# floor set by DMA).
REGION_ROWS = [range(0, 9), range(9, 13), range(13, 18), range(18, 22),
               range(22, 31)]
USED_IDX = np.array([ci * 1024 + r * 32 + w
                     for rows in REGION_ROWS
                     for ci in range(C_IN) for r in rows for w in range(31)],
                    np.int64)
N_USED = len(USED_IDX)         # 2883
K_CHUNKS = 23
F_PACK = K_CHUNKS * 128        # 2944

# group g's features span [lo, hi) in the packed order
_G_LO = [0, 837, 1674]
_G_HI = [1209, 2046, 2883]
GROUP_PAIRS = [list(range(lo // 128, -(-hi // 128)))
               for lo, hi in zip(_G_LO, _G_HI)]
PAIRS = [(g, k) for g in range(3) for k in GROUP_PAIRS[g]]
N_PAIRS = len(PAIRS)           # 30


def _build_nc(repeat=1, mode="full"):
    import concourse.bacc as bacc
    import concourse.mybir as mybir
    from concourse.tile import TileContext

    F16 = mybir.dt.float16
    F32 = mybir.dt.float32
    AF = mybir.ActivationFunctionType

    nc = bacc.Bacc()
    # partition-major pack: [t, p, c, n]; each batch tile is fetched as two
    # chunk-range halves (0:12 / 12:23) so group-0 matmuls can start as soon
    # as the first half lands and group-2 only waits on the second
    xT = nc.declare_dram_parameter(
        "xT", [T_TILES, 128, K_CHUNKS, N_TILE], F16, isOutput=False)
    KA = 12                     # chunks in the first half
    ATp = nc.declare_dram_parameter("ATp", [128, N_PAIRS * M_GROUP], F16, isOutput=False)
    FC1 = nc.declare_dram_parameter("FC1", [M_GROUP, 3 * HIDDEN], F16, isOutput=False)
    FC2 = nc.declare_dram_parameter("FC2", [HIDDEN, OUTPUT], F16, isOutput=False)
    B1 = nc.declare_dram_parameter("B1", [HIDDEN, 1], F32, isOutput=False)
    B2 = nc.declare_dram_parameter("B2", [OUTPUT, 1], F32, isOutput=False)
    OUT = nc.declare_dram_parameter("OUT", [OUTPUT, B_CORE], F32, isOutput=True)

    with TileContext(nc) as tc:
        with tc.tile_pool(name="wpool", bufs=1) as wpool, \
             tc.tile_pool(name="xpool", bufs=6) as xpool, \
             tc.tile_pool(name="ypool", bufs=4) as ypool, \
             tc.tile_pool(name="opool", bufs=1) as opool, \
             tc.tile_pool(name="psy", bufs=4, space="PSUM") as psy, \
             tc.tile_pool(name="psh", bufs=2, space="PSUM") as psh, \
             tc.tile_pool(name="pso", bufs=2, space="PSUM") as pso:

            ats = wpool.tile([128, N_PAIRS * M_GROUP], F16, tag="ats")
            fc1t = wpool.tile([M_GROUP, 3 * HIDDEN], F16, tag="fc1t")
            fc2t = wpool.tile([HIDDEN, OUTPUT], F16, tag="fc2t")
            b1t = wpool.tile([HIDDEN, 1], F32, tag="b1t")
            b2t = wpool.tile([OUTPUT, 1], F32, tag="b2t")
            nc.sync.dma_start(out=ats, in_=ATp[:, :])
            nc.sync.dma_start(out=fc1t, in_=FC1[:, :])
            nc.sync.dma_start(out=fc2t, in_=FC2[:, :])
            nc.sync.dma_start(out=b1t, in_=B1[:, :])
            nc.sync.dma_start(out=b2t, in_=B2[:, :])
            outsb = opool.tile([OUTPUT, B_CORE], F32, tag="outsb")
            if mode == "dma":
                nc.gpsimd.memset(outsb, 0.0)

            if mode == "compute":
                xa_fixed = xpool.tile([128, KA, N_TILE], F16, tag="xa")
                xb_fixed = xpool.tile([128, K_CHUNKS - KA, N_TILE], F16, tag="xb")
                nc.sync.dma_start(out=xa_fixed, in_=xT[0, :, 0:KA, :])
                nc.sync.dma_start(out=xb_fixed, in_=xT[0, :, KA:K_CHUNKS, :])
            for _rep in range(repeat):
                for t in range(T_TILES):
                    if mode == "compute":
                        xa, xb = xa_fixed, xb_fixed
                    else:
                        xa = xpool.tile([128, KA, N_TILE], F16, tag="xa")
                        nc.sync.dma_start(out=xa, in_=xT[t, :, 0:KA, :])
                        xb = xpool.tile([128, K_CHUNKS - KA, N_TILE], F16, tag="xb")
                        nc.sync.dma_start(out=xb, in_=xT[t, :, KA:K_CHUNKS, :])
                    if mode == "dma":
                        continue
                    y2 = []
                    pcnt = 0
                    for g in range(3):
                        ps = psy.tile([M_GROUP, N_TILE], F32, tag="psy")
                        ks = GROUP_PAIRS[g]
                        for idx, k in enumerate(ks):
                            rhs = xa[:, k, :] if k < KA else xb[:, k - KA, :]
                            nc.tensor.matmul(
                                ps,
                                ats[:, pcnt * M_GROUP:(pcnt + 1) * M_GROUP],
                                rhs,
                                start=(idx == 0),
                                stop=(idx == len(ks) - 1),
                            )
                            pcnt += 1
                        yt = ypool.tile([M_GROUP, N_TILE], F16, tag="y2")
                        nc.scalar.activation(yt, ps, AF.Square)
                        y2.append(yt)
                    hp = psh.tile([HIDDEN, N_TILE], F32, tag="psh")
                    for g in range(3):
                        nc.tensor.matmul(
                            hp,
                            fc1t[:, g * HIDDEN:(g + 1) * HIDDEN],
                            y2[g],
                            start=(g == 0),
                            stop=(g == 2),
                        )
                    h2 = ypool.tile([HIDDEN, N_TILE], F16, tag="h2")
                    nc.scalar.activation(h2, hp, AF.Square, bias=b1t)
                    op = pso.tile([OUTPUT, N_TILE], F32, tag="pso")
                    nc.tensor.matmul(op, fc2t, h2, start=True, stop=True)
                    nc.scalar.activation(
                        outsb[:, t * N_TILE:(t + 1) * N_TILE], op, AF.Identity,
                        bias=b2t,
                    )
            nc.sync.dma_start(out=OUT[:, :], in_=outsb)
    nc.finalize()
    return nc


def _prep_weights(conv_w, fc1_w, fc1_b, fc2_w, fc2_b):
    # A[g, local, f]: dense conv matrix split by i-group.
    # local = il*36 + c*9 + j  (i = 3g+il), f = ci*1024 + r*32 + w
    A = np.zeros((3, M_GROUP, C_IN * 1024), np.float32)
    for g in range(3):
        for il in range(3):
            i = 3 * g + il
            for c in range(C_OUT):
                for j in range(H_OUT):
                    row = il * 36 + c * 9 + j
                    for ci in range(C_IN):
                        for ki in range(KERNEL):
                            f0 = ci * 1024 + (3 * i + ki) * 32 + 3 * j
                            A[g, row, f0:f0 + KERNEL] = conv_w[c, ci, ki, :]
    # gather used feature columns (region-major order), zero-pad to F_PACK
    Ap = np.zeros((3, M_GROUP, F_PACK), np.float32)
    Ap[:, :, :N_USED] = A[:, :, USED_IDX]
    # pack the active [128, 108] transposed blocks side by side
    ATp = np.empty((128, N_PAIRS * M_GROUP), np.float16)
    for p, (g, k) in enumerate(PAIRS):
        ATp[:, p * M_GROUP:(p + 1) * M_GROUP] = Ap[g, :, 128 * k:128 * (k + 1)].T
    # fc1 columns permuted to our y-row order: global y row g*108+il*36+c*9+j
    # corresponds to reference flat index c*81 + (3g+il)*9 + j
    gg, ll, cc, jj = np.meshgrid(np.arange(3), np.arange(3), np.arange(C_OUT),
                                 np.arange(H_OUT), indexing="ij")
    orig = (cc * 81 + (3 * gg + ll) * 9 + jj).reshape(-1)
    fc1p = fc1_w[:, orig].T.astype(np.float16)        # [324, 64]
    FC1 = np.empty((M_GROUP, 3 * HIDDEN), np.float16)
    for g in range(3):
        FC1[:, g * HIDDEN:(g + 1) * HIDDEN] = fc1p[g * M_GROUP:(g + 1) * M_GROUP]
    FC2 = np.ascontiguousarray(fc2_w.T.astype(np.float16))  # [64, 10]
    B1 = np.ascontiguousarray(fc1_b.reshape(HIDDEN, 1).astype(np.float32))
    B2 = np.ascontiguousarray(fc2_b.reshape(OUTPUT, 1).astype(np.float32))
    return ATp, FC1, FC2, B1, B2


def _make_in_maps(x, ATp, FC1, FC2, B1, B2):
    in_maps = []
    xf = x.reshape(B_TOT, C_IN * 1024)
    for c in range(N_CORES):
        xs = xf[c * B_CORE:(c + 1) * B_CORE]
        xg = np.zeros((B_CORE, F_PACK), np.float16)
        xg[:, :N_USED] = xs[:, USED_IDX]
        xg = xg.reshape(T_TILES, N_TILE, K_CHUNKS, 128)
        xTc = np.ascontiguousarray(xg.transpose(0, 3, 2, 1))  # [4, 128, 23, 512]
        in_maps.append({"xT": xTc, "ATp": ATp, "FC1": FC1, "FC2": FC2,
                        "B1": B1, "B2": B2})
    return in_maps


def kernel(x, conv_w, fc1_w, fc1_b, fc2_w, fc2_b):
    from concourse.bass_utils import run_bass_kernel_spmd

    x = np.asarray(x, np.float32)
    ATp, FC1, FC2, B1, B2 = _prep_weights(
        np.asarray(conv_w, np.float32), np.asarray(fc1_w, np.float32),
        np.asarray(fc1_b, np.float32), np.asarray(fc2_w, np.float32),
        np.asarray(fc2_b, np.float32))

    in_maps = _make_in_maps(x, ATp, FC1, FC2, B1, B2)

    nc = _build_nc(repeat=1)
    res = run_bass_kernel_spmd(nc, in_maps, list(range(N_CORES)))
    out = np.empty((B_TOT, OUTPUT), np.float32)
    for c in range(N_CORES):
        out[c * B_CORE:(c + 1) * B_CORE] = res.results[c]["OUT"].T
    return out


# revision 18
# speedup vs baseline: 1.1192x; 1.1192x over previous
"""Trainium2 Bass kernel for ConvNet forward (conv7x7s3 -> sq -> fc -> sq -> fc).

Strategy: pure data parallel over 8 NeuronCores (2048 samples each).
The conv is lowered to a block-sparse dense matrix applied via fp16 matmuls
with batch as the moving free dim (N=512). Host pre-transposes x to
feature-major fp16 layout (halves HBM traffic vs fp32; the fp32 PSUM
accumulate keeps the error ~1e-3, well under the 2e-2 gate). Used features
are packed region-major (rows 0-8 | 9-12 | 13-17 | 18-21 | 22-30 of the 31
used rows) so each of the 3 conv output groups reads a contiguous chunk
range: 30 matmul pushes/tile instead of 37 for channel-major packing.
fc1 weights are permuted to match the conv-output row grouping; the
[10, 2048] per-core output is transposed back to [B, 10] on the host.
"""

import numpy as np

for _p in ("/opt/trn_rl_repo", "/root/.axon_site/_ro/trn_rl_repo"):
    try:
        import concourse  # noqa: F401
        break
    except ImportError:
        import sys
        if _p not in sys.path:
            sys.path.insert(0, _p)

# network constants
KERNEL, STRIDE = 7, 3
C_IN, C_OUT = 3, 4
HIDDEN, OUTPUT = 64, 10
H_OUT = 9                      # (32-7)//3 + 1
B_TOT, N_CORES = 16384, 8
B_CORE = B_TOT // N_CORES      # 2048
N_TILE = 512                   # batch tile (matmul moving free dim)
T_TILES = B_CORE // N_TILE     # 4
M_GROUP = 3 * C_OUT * H_OUT    # 108 outputs per i-group (3 rows x 4 ch x 9 cols)

# Only 31x31 of each 32x32 input image is read by the conv (stride 3, k=7).
# i-group g in {0,1,2} covers output rows i in {3g,3g+1,3g+2} and needs input
# rows r in [9g, 9g+12]. Pack the 3*31*31 = 2883 used features region-major —
# rows [0,9) g0-only | [9,13) shared g0/g1 | [13,18) g1-only | [18,22) shared
# g1/g2 | [22,31) g2-only — so each group's features are one contiguous span
# and its chunk list is a contiguous range (30 total pushes, near the 23-chunk
# floor set by DMA).
REGION_ROWS = [range(0, 9), range(9, 13), range(13, 18), range(18, 22),
               range(22, 31)]
USED_IDX = np.array([ci * 1024 + r * 32 + w
                     for rows in REGION_ROWS
                     for ci in range(C_IN) for r in rows for w in range(31)],
                    np.int64)
N_USED = len(USED_IDX)         # 2883
K_CHUNKS = 23
F_PACK = K_CHUNKS * 128        # 2944

# group g's features span [lo, hi) in the packed order
_G_LO = [0, 837, 1674]
_G_HI = [1209, 2046, 2883]
GROUP_PAIRS = [list(range(lo // 128, -(-hi // 128)))
               for lo, hi in zip(_G_LO, _G_HI)]
PAIRS = [(g, k) for g in range(3) for k in GROUP_PAIRS[g]]
N_PAIRS = len(PAIRS)           # 30


def _build_nc(repeat=1, mode="full"):
    import concourse.bacc as bacc
    import concourse.mybir as mybir
    from concourse.tile import TileContext

    F16 = mybir.dt.float16
    F32 = mybir.dt.float32
    AF = mybir.ActivationFunctionType

    nc = bacc.Bacc()
    # partition-major pack: [t, p, c, n] so each partition's DMA read is a
    # single 23.5 KB contiguous run
    xT = nc.declare_dram_parameter(
        "xT", [T_TILES, 128, K_CHUNKS, N_TILE], F16, isOutput=False)
    ATp = nc.declare_dram_parameter("ATp", [128, N_PAIRS * M_GROUP], F16, isOutput=False)
    FC1 = nc.declare_dram_parameter("FC1", [M_GROUP, 3 * HIDDEN], F16, isOutput=False)
    FC2 = nc.declare_dram_parameter("FC2", [HIDDEN, OUTPUT], F16, isOutput=False)
    B1 = nc.declare_dram_parameter("B1", [HIDDEN, 1], F32, isOutput=False)
    B2 = nc.declare_dram_parameter("B2", [OUTPUT, 1], F32, isOutput=False)
    OUT = nc.declare_dram_parameter("OUT", [OUTPUT, B_CORE], F32, isOutput=True)

    with TileContext(nc) as tc:
        with tc.tile_pool(name="wpool", bufs=1) as wpool, \
             tc.tile_pool(name="xpool", bufs=3) as xpool, \
             tc.tile_pool(name="ypool", bufs=4) as ypool, \
             tc.tile_pool(name="opool", bufs=1) as opool, \
             tc.tile_pool(name="psy", bufs=4, space="PSUM") as psy, \
             tc.tile_pool(name="psh", bufs=2, space="PSUM") as psh, \
             tc.tile_pool(name="pso", bufs=2, space="PSUM") as pso:

            ats = wpool.tile([128, N_PAIRS * M_GROUP], F16, tag="ats")
            fc1t = wpool.tile([M_GROUP, 3 * HIDDEN], F16, tag="fc1t")
            fc2t = wpool.tile([HIDDEN, OUTPUT], F16, tag="fc2t")
            b1t = wpool.tile([HIDDEN, 1], F32, tag="b1t")
            b2t = wpool.tile([OUTPUT, 1], F32, tag="b2t")
            nc.sync.dma_start(out=ats, in_=ATp[:, :])
            nc.sync.dma_start(out=fc1t, in_=FC1[:, :])
            nc.sync.dma_start(out=fc2t, in_=FC2[:, :])
            nc.sync.dma_start(out=b1t, in_=B1[:, :])
            nc.sync.dma_start(out=b2t, in_=B2[:, :])
            outsb = opool.tile([OUTPUT, B_CORE], F32, tag="outsb")
            if mode == "dma":
                nc.gpsimd.memset(outsb, 0.0)

            if mode == "compute":
                xt_fixed = xpool.tile([128, K_CHUNKS, N_TILE], F16, tag="xt")
                nc.sync.dma_start(out=xt_fixed, in_=xT[0])
            for _rep in range(repeat):
                for t in range(T_TILES):
                    if mode == "compute":
                        xt = xt_fixed
                    else:
                        xt = xpool.tile([128, K_CHUNKS, N_TILE], F16, tag="xt")
                        nc.sync.dma_start(out=xt, in_=xT[t])
                    if mode == "dma":
                        continue
                    y2 = []
                    pcnt = 0
                    for g in range(3):
                        ps = psy.tile([M_GROUP, N_TILE], F32, tag="psy")
                        ks = GROUP_PAIRS[g]
                        for idx, k in enumerate(ks):
                            nc.tensor.matmul(
                                ps,
                                ats[:, pcnt * M_GROUP:(pcnt + 1) * M_GROUP],
                                xt[:, k, :],
                                start=(idx == 0),
                                stop=(idx == len(ks) - 1),
                            )
                            pcnt += 1
                        yt = ypool.tile([M_GROUP, N_TILE], F16, tag="y2")
                        nc.scalar.activation(yt, ps, AF.Square)
                        y2.append(yt)
                    hp = psh.tile([HIDDEN, N_TILE], F32, tag="psh")
                    for g in range(3):
                        nc.tensor.matmul(
                            hp,
                            fc1t[:, g * HIDDEN:(g + 1) * HIDDEN],
                            y2[g],
                            start=(g == 0),
                            stop=(g == 2),
                        )
                    h2 = ypool.tile([HIDDEN, N_TILE], F16, tag="h2")
                    nc.scalar.activation(h2, hp, AF.Square, bias=b1t)
                    op = pso.tile([OUTPUT, N_TILE], F32, tag="pso")
                    nc.tensor.matmul(op, fc2t, h2, start=True, stop=True)
                    nc.scalar.activation(
                        outsb[:, t * N_TILE:(t + 1) * N_TILE], op, AF.Identity,
                        bias=b2t,
                    )
            nc.sync.dma_start(out=OUT[:, :], in_=outsb)
    nc.finalize()
    return nc


def _prep_weights(conv_w, fc1_w, fc1_b, fc2_w, fc2_b):
    # A[g, local, f]: dense conv matrix split by i-group.
    # local = il*36 + c*9 + j  (i = 3g+il), f = ci*1024 + r*32 + w
    A = np.zeros((3, M_GROUP, C_IN * 1024), np.float32)
    for g in range(3):
        for il in range(3):
            i = 3 * g + il
            for c in range(C_OUT):
                for j in range(H_OUT):
                    row = il * 36 + c * 9 + j
                    for ci in range(C_IN):
                        for ki in range(KERNEL):
                            f0 = ci * 1024 + (3 * i + ki) * 32 + 3 * j
                            A[g, row, f0:f0 + KERNEL] = conv_w[c, ci, ki, :]
    # gather used feature columns (region-major order), zero-pad to F_PACK
    Ap = np.zeros((3, M_GROUP, F_PACK), np.float32)
    Ap[:, :, :N_USED] = A[:, :, USED_IDX]
    # pack the active [128, 108] transposed blocks side by side
    ATp = np.empty((128, N_PAIRS * M_GROUP), np.float16)
    for p, (g, k) in enumerate(PAIRS):
        ATp[:, p * M_GROUP:(p + 1) * M_GROUP] = Ap[g, :, 128 * k:128 * (k + 1)].T
    # fc1 columns permuted to our y-row order: global y row g*108+il*36+c*9+j
    # corresponds to reference flat index c*81 + (3g+il)*9 + j
    gg, ll, cc, jj = np.meshgrid(np.arange(3), np.arange(3), np.arange(C_OUT),
                                 np.arange(H_OUT), indexing="ij")
    orig = (cc * 81 + (3 * gg + ll) * 9 + jj).reshape(-1)
    fc1p = fc1_w[:, orig].T.astype(np.float16)        # [324, 64]
    FC1 = np.empty((M_GROUP, 3 * HIDDEN), np.float16)
    for g in range(3):
        FC1[:, g * HIDDEN:(g + 1) * HIDDEN] = fc1p[g * M_GROUP:(g + 1) * M_GROUP]
    FC2 = np.ascontiguousarray(fc2_w.T.astype(np.float16))  # [64, 10]
    B1 = np.ascontiguousarray(fc1_b.reshape(HIDDEN, 1).astype(np.float32))
    B2 = np.ascontiguousarray(fc2_b.reshape(OUTPUT, 1).astype(np.float32))
    return ATp, FC1, FC2, B1, B2


def _make_in_maps(x, ATp, FC1, FC2, B1, B2):
    in_maps = []
    xf = x.reshape(B_TOT, C_IN * 1024)
    for c in range(N_CORES):
        xs = xf[c * B_CORE:(c + 1) * B_CORE]
        xg = np.zeros((B_CORE, F_PACK), np.float16)
        xg[:, :N_USED] = xs[:, USED_IDX]
        xg = xg.reshape(T_TILES, N_TILE, K_CHUNKS, 128)
        xTc = np.ascontiguousarray(xg.transpose(0, 3, 2, 1))  # [4, 128, 23, 512]
        in_maps.append({"xT": xTc, "ATp": ATp, "FC1": FC1, "FC2": FC2,
                        "B1": B1, "B2": B2})
    return in_maps


def kernel(x, conv_w, fc1_w, fc1_b, fc2_w, fc2_b):
    from concourse.bass_utils import run_bass_kernel_spmd

    x = np.asarray(x, np.float32)
    ATp, FC1, FC2, B1, B2 = _prep_weights(
        np.asarray(conv_w, np.float32), np.asarray(fc1_w, np.float32),
        np.asarray(fc1_b, np.float32), np.asarray(fc2_w, np.float32),
        np.asarray(fc2_b, np.float32))

    in_maps = _make_in_maps(x, ATp, FC1, FC2, B1, B2)

    nc = _build_nc(repeat=1)
    res = run_bass_kernel_spmd(nc, in_maps, list(range(N_CORES)))
    out = np.empty((B_TOT, OUTPUT), np.float32)
    for c in range(N_CORES):
        out[c * B_CORE:(c + 1) * B_CORE] = res.results[c]["OUT"].T
    return out
